# revision 1
# baseline (speedup 1.0000x reference)
"""Trainium2 Bass kernel for nn_AttentionBlock (B=32, C=1024, H=W=32, nh=1).

Reference computation (per batch b, with S = H*W = 1024):
    qkv = w_qkv @ x_b            # [3C, S], 1x1 conv == channel matmul
    q, k, v = split(qkv)
    logits[t,s] = (q[:,t] . k[:,s]) / sqrt(C)
    attn = softmax_s(logits)
    h[t,s] = attn[t,s] * sum_c v[c,s]
    out = w_proj @ h + b_proj + x_b

Algebraic simplifications (all weight-only, precomputed on host):
  * logits = x^T (M x) with M = Wq^T Wk  -> q/k never materialized.
  * sum_c v[c,s] = (sum_c Wv[c,:]) . x[:,s] = vs, computed on the
    vector/scalar engines + one ones-matmul (partition reduction).
  * h = attn .* (1 vs^T).

Precision: stages A/B (everything feeding the softmax) run in float32r
(single-pass fp32 matmul, 11 mantissa bits). Stage C (projection of the
small attention term) runs in bf16 — its output is ~13% of |out|, so the
extra rounding is negligible. The residual adds exact fp32 x.

Sharding: data-parallel over batch, 4 batches per core on 8 cores.
"""

import os
import sys

import numpy as np

for _p in ("/opt/trn_rl_repo", "/opt/pypackages"):
    if _p not in sys.path:
        sys.path.insert(0, _p)

import ml_dtypes

import concourse.bass as bass
import concourse.tile as tile
from concourse import bacc, mybir
from concourse.bass_utils import run_bass_kernel_spmd
from concourse.tile_rust import add_dep_helper

B, C, HH, WW = 32, 1024, 32, 32
S = HH * WW          # 1024 spatial positions
P = 128              # partitions
KC = C // P          # 8 chunks along channel dim
TC = S // P          # 8 chunks along spatial (t) dim
NN = 512             # matmul moving free dim
NCH = S // NN        # 2 free-dim halves
N_CORES = 8
BPC = B // N_CORES   # batches per core
SCALE = 1.0 / np.sqrt(float(C))  # folded into the exp

f32 = mybir.dt.float32
f32r = mybir.dt.float32r
bf16 = mybir.dt.bfloat16

# dtype config: "all_f32r" (default), "f32r_cbf16" (A/B f32r + C bf16),
# "bf16" (everything bf16 — fastest, ~1.4e-3 absmax error)
MM_CFG = os.environ.get("KERNEL_MM_CFG", "all_f32r")


def _cfg(name):
    if name == "bf16":
        return bf16, bf16
    if name == "all_f32r":
        return f32r, f32r
    return f32r, bf16  # default: A/B f32r, C bf16


def build_nc(bpc: int = BPC, cfg_name: str | None = None):
    abdt, cdt = _cfg(cfg_name or MM_CFG)
    nc = bacc.Bacc(
        "TRN2",
        target_bir_lowering=False,
        debug=False,
        enable_asserts=False,
    )

    x_d = nc.dram_tensor("x", [bpc, C, S], abdt, kind="ExternalInput")
    xf_d = nc.dram_tensor("xf", [bpc, C, S], f32, kind="ExternalInput")
    # weight stripes pre-arranged on host: [chunk, c, 128]
    mt_d = nc.dram_tensor("mt", [KC, C, P], abdt, kind="ExternalInput")
    wpt_d = nc.dram_tensor("wpt", [KC, C, P], cdt, kind="ExternalInput")
    wvs_d = nc.dram_tensor("wvs", [C], f32, kind="ExternalInput")
    ones_d = nc.dram_tensor("ones", [P, P], abdt, kind="ExternalInput")
    bp_d = nc.dram_tensor("bp", [C], f32, kind="ExternalInput")
    out_d = nc.dram_tensor("out", [bpc, C, S], f32, kind="ExternalOutput")

    big = cdt != bf16  # all-f32r needs tighter pools to fit SBUF
    with tile.TileContext(nc) as tc:
        with (
            tc.tile_pool(name="weights", bufs=1) as wpool,
            tc.tile_pool(name="xc", bufs=11 if big else 10) as xpool,
            tc.tile_pool(name="xf", bufs=1 if big else 4) as xfpool,
            tc.tile_pool(name="y", bufs=1) as ypool,
            tc.tile_pool(name="h", bufs=9 if big else 10) as hpool,
            tc.tile_pool(name="vsb", bufs=1 if big else 2) as vpool,
            tc.tile_pool(name="vacc", bufs=1 if big else 2) as vaccpool,
            tc.tile_pool(name="vtmp", bufs=4 if big else 5) as vtmppool,
            tc.tile_pool(name="osb", bufs=3 if big else 4) as opool,
            tc.tile_pool(name="small", bufs=8) as spool,
            tc.tile_pool(name="psA", bufs=3, space="PSUM") as psA,
            tc.tile_pool(name="psB", bufs=2, space="PSUM") as psB,
            tc.tile_pool(name="psC", bufs=3, space="PSUM") as psC,
        ):
            # ---- small resident weights first (cheap DMAs) ----
            wvs_sb = wpool.tile([P, KC], f32, tag="wvs")
            nc.sync.dma_start(wvs_sb[:], wvs_d.rearrange("(ko ki) -> ki ko", ki=P))
            bp_sb = wpool.tile([P, KC], f32, tag="bp")
            nc.sync.dma_start(bp_sb[:], bp_d.rearrange("(o p) -> p o", p=P))
            ones_sb = wpool.tile([P, P], abdt, tag="ones")
            nc.sync.dma_start(ones_sb[:], ones_d[:, :])
            # warm the PE clock (HAM) with throwaway matmuls while the
            # first batch's weights/x DMAs are in flight
            wu = psA.tile([P, NN], f32, tag="psA")
            for _ in range(25):
                nc.tensor.matmul(
                    wu[:, 0:64], ones_sb[:], ones_sb[:, 0:64],
                    start=True, stop=True,
                )
            wpt_sb = wpool.tile([P, TC, C], cdt, tag="wpt")
            mt_sb = wpool.tile([P, KC, C], abdt, tag="mt")

            for b in range(bpc):
                # ---- load x chunks; order matches first-use order ----
                xc = []
                for k in range(KC):
                    t = xpool.tile([P, S], abdt, tag="xc")
                    xc.append(t)
                if b == 0:
                    # Critical startup set: mt stripe 0 + x half 0 — the
                    # first psum group's inputs. Everything else is chained
                    # behind them so concurrent DMA queues don't dilute the
                    # bandwidth the first matmuls are waiting on.
                    crit = [
                        nc.sync.dma_start(
                            mt_sb[:, :, 0:P],
                            mt_d[0].rearrange("(ko ki) m -> ki ko m", ki=P),
                        )
                    ]
                    for k in range(KC):
                        crit.append(
                            nc.sync.dma_start(
                                xc[k][:, 0:NN], x_d[b, k * P : (k + 1) * P, 0:NN]
                            )
                        )
                    gate = crit[-1].ins
                    noncrit = []
                    for k in range(KC):
                        noncrit.append(
                            nc.sync.dma_start(
                                xc[k][:, NN:S], x_d[b, k * P : (k + 1) * P, NN:S]
                            )
                        )
                    for mc in range(1, KC):
                        noncrit.append(
                            nc.sync.dma_start(
                                mt_sb[:, :, mc * P : (mc + 1) * P],
                                mt_d[mc].rearrange("(ko ki) m -> ki ko m", ki=P),
                            )
                        )
                    for inst in noncrit:
                        add_dep_helper(
                            inst.ins, gate, sync=True,
                            reason="startup: critical DMAs first",
                        )
                else:
                    for k in range(KC):
                        nc.sync.dma_start(xc[k][:], x_d[b, k * P : (k + 1) * P, :])

                # ---- stage A2a: vacc[p,s] = sum_k wvs[k*128+p] * x[k][p,s]
                # products on ACT; pairwise-tree adds split over DVE and
                # GPSIMD so the (busy) vector engine isn't the serializer.
                # PE only does the final 128-partition ones-matmul.
                vacc = vaccpool.tile([P, S], abdt, tag="vacc")

                def _vprod(k):
                    vt = vtmppool.tile([P, S], f32, tag="vtmp")
                    nc.scalar.activation(
                        vt[:], xc[k][:],
                        mybir.ActivationFunctionType.Copy,
                        scale=wvs_sb[:, k : k + 1],
                    )
                    return vt

                p0, p1 = _vprod(0), _vprod(1)
                nc.vector.tensor_tensor(p0[:], p0[:], p1[:], mybir.AluOpType.add)
                p2, p3 = _vprod(2), _vprod(3)
                nc.gpsimd.tensor_tensor(p2[:], p2[:], p3[:], mybir.AluOpType.add)
                nc.gpsimd.tensor_tensor(p0[:], p0[:], p2[:], mybir.AluOpType.add)
                p4, p5 = _vprod(4), _vprod(5)
                nc.vector.tensor_tensor(p4[:], p4[:], p5[:], mybir.AluOpType.add)
                p6, p7 = _vprod(6), _vprod(7)
                nc.gpsimd.tensor_tensor(p6[:], p6[:], p7[:], mybir.AluOpType.add)
                nc.vector.tensor_tensor(p4[:], p4[:], p6[:], mybir.AluOpType.add)
                nc.vector.tensor_tensor(vacc[:], p0[:], p4[:], mybir.AluOpType.add)

                # ---- stage A: y = M x  (y[c',s]) ----
                y_sb = ypool.tile([P, KC, S], abdt, tag="y")
                for mc in range(KC):
                    for n in range(NCH):
                        ps = psA.tile([P, NN], f32, tag="psA")
                        for k in range(KC):
                            nc.tensor.matmul(
                                ps[:],
                                mt_sb[:, k, mc * P : (mc + 1) * P],
                                xc[k][:, n * NN : (n + 1) * NN],
                                start=(k == 0),
                                stop=(k == KC - 1),
                            )
                        nc.any.tensor_copy(
                            out=y_sb[:, mc, n * NN : (n + 1) * NN], in_=ps[:]
                        )

                # ---- stage A2b: vs broadcast via ones-matmul ----
                vsb = vpool.tile([P, S], cdt, tag="vsb")
                for n in range(NCH):
                    psv = psA.tile([P, NN], f32, tag="psA")
                    nc.tensor.matmul(
                        psv[:], ones_sb[:], vacc[:, n * NN : (n + 1) * NN],
                        start=True, stop=True,
                    )
                    nc.any.tensor_copy(out=vsb[:, n * NN : (n + 1) * NN], in_=psv[:])

                if b == 0:
                    # proj weights not needed until stage C; loading them here
                    # keeps the critical-path DMAs (mt, x) uncontended.
                    for oc in range(KC):
                        nc.sync.dma_start(
                            wpt_sb[:, :, oc * P : (oc + 1) * P],
                            wpt_d[oc].rearrange("(ko ki) m -> ki ko m", ki=P),
                        )

                # ---- stage B: logits tiles, fused softmax * vs ----
                hts = []
                for tt in range(TC):
                    e = hpool.tile([P, S], cdt, tag="h")
                    rsh = []
                    for n in range(NCH):
                        psl = psB.tile([P, NN], f32, tag="psB")
                        for k in range(KC):
                            nc.tensor.matmul(
                                psl[:],
                                xc[k][:, tt * P : (tt + 1) * P],
                                y_sb[:, k, n * NN : (n + 1) * NN],
                                start=(k == 0),
                                stop=(k == KC - 1),
                            )
                        # e-half = exp(logits / sqrt(C)); rs = partial row sum
                        rs = spool.tile([P, 1], f32, tag="rs")
                        nc.scalar.activation(
                            e[:, n * NN : (n + 1) * NN], psl[:],
                            mybir.ActivationFunctionType.Exp,
                            scale=float(SCALE), accum_out=rs[:],
                        )
                        rsh.append(rs)
                    rst = spool.tile([P, 1], f32, tag="rst")
                    nc.vector.tensor_tensor(
                        rst[:], rsh[0][:], rsh[1][:], mybir.AluOpType.add
                    )
                    rcp = spool.tile([P, 1], f32, tag="rcp")
                    nc.vector.reciprocal(rcp[:], rst[:])
                    # normalize rows (per-partition scalar) on ACT
                    nc.scalar.activation(
                        e[:], e[:], mybir.ActivationFunctionType.Copy,
                        scale=rcp[:],
                    )
                    # h = attn * vs  (vs broadcast over partitions via vsb)
                    nc.vector.tensor_tensor(
                        e[:], e[:], vsb[:], mybir.AluOpType.mult
                    )
                    hts.append(e)

                # ---- stage C: out = w_proj @ h + x + b ----
                # last batch: no stage-A/B work follows, so spread C's psum
                # tiles over every pool — deeper pipelining at the tail
                cpools = (
                    [(psC, "psC"), (psA, "psA"), (psB, "psB")]
                    if b == bpc - 1
                    else [(psC, "psC")]
                )
                for oc in range(KC):
                    for n in range(NCH):
                        cp, ctag = cpools[(oc * NCH + n) % len(cpools)]
                        pso = cp.tile([P, NN], f32, tag=ctag)
                        for tt in range(TC):
                            nc.tensor.matmul(
                                pso[:],
                                wpt_sb[:, tt, oc * P : (oc + 1) * P],
                                hts[tt][:, n * NN : (n + 1) * NN],
                                start=(tt == 0),
                                stop=(tt == TC - 1),
                            )
                        if big:
                            # residual from the resident f32r x (saves 4 MiB
                            # of DMA per batch; costs ~1e-4 absmax rounding)
                            x_res = xc[oc][:, n * NN : (n + 1) * NN].bitcast(f32)
                        else:
                            xf_t = xfpool.tile([P, NN], f32, tag="xf")
                            nc.sync.dma_start(
                                xf_t[:],
                                xf_d[b, oc * P : (oc + 1) * P, n * NN : (n + 1) * NN],
                            )
                            x_res = xf_t[:]
                        osb = opool.tile([P, NN], f32, tag="osb")
                        # copy+bias off PSUM, alternating engines so neither
                        # ACT nor DVE serializes the PSUM-bank release
                        if (oc + n) % 2 == 0:
                            nc.scalar.activation(
                                osb[:], pso[:],
                                mybir.ActivationFunctionType.Identity,
                                bias=bp_sb[:, oc : oc + 1],
                            )
                        else:
                            nc.vector.tensor_scalar(
                                osb[:], pso[:], bp_sb[:, oc : oc + 1], None,
                                mybir.AluOpType.add,
                            )
                        nc.vector.tensor_tensor(
                            osb[:], osb[:], x_res, mybir.AluOpType.add
                        )
                        nc.sync.dma_start(
                            out_d[b, oc * P : (oc + 1) * P, n * NN : (n + 1) * NN],
                            osb[:],
                        )
    nc.compile()
    return nc


def _round_f32r(a):
    """Round fp32 to float32r (11-bit mantissa, round-to-nearest-even)."""
    u = np.ascontiguousarray(a.astype(np.float32)).view(np.uint32)
    lsb = (u >> np.uint32(12)) & np.uint32(1)
    r = (u + np.uint32(0x7FF) + lsb) & np.uint32(0xFFFFF000)
    return r.view(np.float32)


def _to_dt(a, dt):
    if dt == bf16:
        return np.ascontiguousarray(a).astype(ml_dtypes.bfloat16)
    if dt == f32r:
        return _round_f32r(a)
    return np.ascontiguousarray(a).astype(np.float32)


def _host_prep(w_qkv, w_proj, b_proj, cfg_name):
    abdt, cdt = _cfg(cfg_name)
    wq = w_qkv[0:C].astype(np.float64)
    wk = w_qkv[C : 2 * C].astype(np.float64)
    wv = w_qkv[2 * C : 3 * C]
    # lhsT for y-matmul: MT[c, c'] = M[c', c],  M = Wq^T Wk  =>  MT = Wk^T Wq
    mt = np.ascontiguousarray(wk.T @ wq).astype(np.float32)
    wvs = wv.sum(axis=0, dtype=np.float64).astype(np.float32)
    wpt = np.ascontiguousarray(w_proj.T).astype(np.float32)
    # stripe layout [chunk, c, 128]
    mt_s = np.ascontiguousarray(mt.reshape(C, KC, P).transpose(1, 0, 2))
    wpt_s = np.ascontiguousarray(wpt.reshape(C, KC, P).transpose(1, 0, 2))
    return _to_dt(mt_s, abdt), _to_dt(wpt_s, cdt), wvs, b_proj.astype(np.float32)


_NC_CACHE = {}


def _get_nc(bpc=BPC, cfg_name=None):
    key = (bpc, cfg_name or MM_CFG)
    if key not in _NC_CACHE:
        _NC_CACHE[key] = build_nc(bpc, cfg_name)
    return _NC_CACHE[key]


def kernel(x, w_qkv, w_proj, b_proj, _trace=False):
    cfg_name = MM_CFG
    abdt, _ = _cfg(cfg_name)
    x = np.asarray(x, dtype=np.float32)
    mt, wpt, wvs, bp = _host_prep(
        np.asarray(w_qkv, np.float32),
        np.asarray(w_proj, np.float32),
        np.asarray(b_proj, np.float32),
        cfg_name,
    )
    xr_full = x.reshape(B, C, S)
    x_mm = _to_dt(xr_full, abdt)
    in_maps = []
    for c in range(N_CORES):
        sl = slice(c * BPC, (c + 1) * BPC)
        in_maps.append(
            {
                "x": np.ascontiguousarray(x_mm[sl]),
                "xf": np.ascontiguousarray(xr_full[sl]),
                "mt": mt,
                "wpt": wpt,
                "wvs": wvs,
                "ones": _to_dt(np.ones((P, P), np.float32), abdt),
                "bp": bp,
            }
        )
    nc = _get_nc(BPC, cfg_name)
    res = run_bass_kernel_spmd(
        nc, in_maps, core_ids=list(range(N_CORES)), trace=_trace
    )
    out = np.concatenate([r["out"] for r in res.results], axis=0)
    out = out.reshape(B, C, HH, WW)
    if _trace:
        kernel.last_results = res
    return out



# revision 6
# speedup vs baseline: 1.2201x; 1.2201x over previous
"""Trainium2 Bass kernel for nn_AttentionBlock (B=32, C=1024, H=W=32, nh=1).

Reference computation (per batch b, with S = H*W = 1024):
    qkv = w_qkv @ x_b            # [3C, S], 1x1 conv == channel matmul
    q, k, v = split(qkv)
    logits[t,s] = (q[:,t] . k[:,s]) / sqrt(C)
    attn = softmax_s(logits)
    h[t,s] = attn[t,s] * sum_c v[c,s]
    out = w_proj @ h + b_proj + x_b

Algebraic simplifications (weight-only, precomputed on host):
  * logits = x^T (M x) with M = Wq^T Wk  -> q/k never materialized.
  * sum_c v[c,s] = (sum_c Wv[c,:]) . x[:,s] = vs  (ACT products + tree adds
    + one ones-matmul partition reduction).
  * h = attn .* (1 vs^T); softmax row-normalization is folded into the
    projection weights: out = ((Wp^T/rs) @ exp(l)) .* vs + b + x, so the
    exp's output never needs a second normalization pass.

Precision:
  * Stage A (y = 16*M^T x): fp8 e4m3 with DoubleRow (256-deep contraction
    per matmul -> 2x tensor-engine throughput). M is pre-scaled by 16 on
    host so its entries sit in e4m3's normal range; the 1/16 is folded
    into the exp scale.
  * Stages B (logits) and C (projection): bf16 (fp8 would blow the 2e-2
    error budget; measured sim rel-err of this config is ~1.3e-2 vs
    3.2e-2 for fp8-everywhere).
  * PSUM accumulation is fp32 everywhere; residual adds bf16-rounded x.

Sharding: data-parallel over batch, 4 batches per core on 8 cores.
"""

import os
import sys

import numpy as np

for _p in ("/opt/trn_rl_repo", "/opt/pypackages"):
    if _p not in sys.path:
        sys.path.insert(0, _p)

import ml_dtypes

import concourse.bass as bass
import concourse.tile as tile
from concourse import bacc, mybir
from concourse.bass_utils import run_bass_kernel_spmd
from concourse.tile_rust import add_dep_helper

B, C, HH, WW = 32, 1024, 32, 32
S = HH * WW          # 1024 spatial positions
P = 128              # partitions
KC = C // P          # 8 chunks along channel dim
TC = S // P          # 8 chunks along spatial (t) dim
QC = C // 256        # 4 DoubleRow chunks along contraction dim
NN = 512             # matmul moving free dim
NCH = S // NN        # 2 free-dim halves
N_CORES = 8
BPC = B // N_CORES   # batches per core
A_SCALE = 16.0       # host pre-scale of M for fp8 range
SCALE = 1.0 / (np.sqrt(float(C)) * A_SCALE)  # folded into the exp

f32 = mybir.dt.float32
f32r = mybir.dt.float32r
bf16 = mybir.dt.bfloat16
fp8 = mybir.dt.float8e4

N_WARMUP = int(os.environ.get("KERNEL_WARMUP", "30"))


def build_nc(bpc: int = BPC):
    nc = bacc.Bacc(
        "TRN2",
        target_bir_lowering=False,
        debug=False,
        enable_asserts=False,
    )

    # x in fp8 DoubleRow layout [q, p, i, s]: channel c = q*256 + i*128 + p
    x8_d = nc.dram_tensor("x8", [bpc, QC, P, 2, S], fp8, kind="ExternalInput")
    # x in bf16, plain chunk layout [k, p, s]: c = k*128 + p
    xbf_d = nc.dram_tensor("xbf", [bpc, KC, P, S], bf16, kind="ExternalInput")
    # A16[mc][p][q][i][m]: lhsT stripe layout for stage A (fp8, 16*Wk^T Wq)
    a16_d = nc.dram_tensor("a16", [KC, P, QC, 2, P], fp8, kind="ExternalInput")
    # w_proj^T stripes: [tt][p][o]
    wpt_d = nc.dram_tensor("wpt", [TC, P, C], bf16, kind="ExternalInput")
    wvs_d = nc.dram_tensor("wvs", [C], f32, kind="ExternalInput")
    ones_d = nc.dram_tensor("ones", [P, P], f32r, kind="ExternalInput")
    bp_d = nc.dram_tensor("bp", [C], f32, kind="ExternalInput")
    out_d = nc.dram_tensor("out", [bpc, C, S], f32, kind="ExternalOutput")

    with tile.TileContext(nc) as tc:
        with (
            tc.tile_pool(name="weights", bufs=1) as wpool,
            tc.tile_pool(name="x8", bufs=2) as x8pool,
            tc.tile_pool(name="xbf", bufs=2) as xbfpool,
            tc.tile_pool(name="xpb", bufs=1) as xpbpool,
            tc.tile_pool(name="y", bufs=1) as ypool,
            tc.tile_pool(name="e", bufs=1) as epool,
            tc.tile_pool(name="wpts", bufs=1) as wptspool,
            tc.tile_pool(name="vacc", bufs=1) as vaccpool,
            tc.tile_pool(name="vsb", bufs=2) as vpool,
            tc.tile_pool(name="vtmp", bufs=4) as vtmppool,
            tc.tile_pool(name="osb", bufs=4) as opool,
            tc.tile_pool(name="small", bufs=40) as spool,
            tc.tile_pool(name="psA", bufs=3, space="PSUM") as psA,
            tc.tile_pool(name="psB", bufs=2, space="PSUM") as psB,
            tc.tile_pool(name="psC", bufs=3, space="PSUM") as psC,
        ):
            # ---- small resident weights first (cheap DMAs) ----
            wvs_sb = wpool.tile([P, KC], f32, tag="wvs")
            nc.sync.dma_start(wvs_sb[:], wvs_d.rearrange("(ko ki) -> ki ko", ki=P))
            bp_sb = wpool.tile([P, KC], f32, tag="bp")
            nc.sync.dma_start(bp_sb[:], bp_d.rearrange("(o p) -> p o", p=P))
            ones_sb = wpool.tile([P, P], f32r, tag="ones")
            nc.sync.dma_start(ones_sb[:], ones_d[:, :])
            # warm the PE clock (HAM) with throwaway matmuls while the
            # first batch's weights/x DMAs are in flight
            wu = psA.tile([P, NN], f32, tag="psA")
            for _ in range(N_WARMUP):
                nc.tensor.matmul(
                    wu[:, 0:64], ones_sb[:], ones_sb[:, 0:64],
                    start=True, stop=True,
                )
            a16_sb = wpool.tile([P, QC, 2, KC, P], fp8, tag="a16")
            wpt_sb = wpool.tile([P, TC, C], bf16, tag="wpt")

            for b in range(bpc):
                x8t = x8pool.tile([P, QC, 2, S], fp8, tag="x8")
                xbf = xbfpool.tile([P, KC, S], bf16, tag="xbf")
                if b == 0:
                    # Critical startup set: A16 stripe 0 + x8 first halves —
                    # the first psum group's inputs. Everything else chains
                    # behind so concurrent DMA queues don't dilute the
                    # bandwidth the first matmuls wait on.
                    crit = [
                        nc.sync.dma_start(
                            a16_sb[:, :, :, 0, :],
                            a16_d[0].rearrange("p q i m -> p q i m"),
                        )
                    ]
                    for q in range(QC):
                        crit.append(
                            nc.sync.dma_start(
                                x8t[:, q, :, 0:NN], x8_d[b, q, :, :, 0:NN]
                            )
                        )
                    gate = crit[-1].ins
                    noncrit = []
                    for mc in range(1, KC):
                        noncrit.append(
                            nc.sync.dma_start(
                                a16_sb[:, :, :, mc, :],
                                a16_d[mc].rearrange("p q i m -> p q i m"),
                            )
                        )
                    for q in range(QC):
                        noncrit.append(
                            nc.sync.dma_start(
                                x8t[:, q, :, NN:S], x8_d[b, q, :, :, NN:S]
                            )
                        )
                    for k in range(KC):
                        noncrit.append(
                            nc.sync.dma_start(xbf[:, k, :], xbf_d[b, k])
                        )
                    for inst in noncrit:
                        add_dep_helper(
                            inst.ins, gate, sync=True,
                            reason="startup: critical DMAs first",
                        )
                else:
                    for q in range(QC):
                        nc.sync.dma_start(x8t[:, q, :, :], x8_d[b, q])
                    for k in range(KC):
                        nc.sync.dma_start(xbf[:, k, :], xbf_d[b, k])

                # ---- vacc[p,s] = sum_k wvs[k*128+p] * x[k][p,s] ----
                # products on ACT; tree adds split over DVE and GPSIMD.
                vacc = vaccpool.tile([P, S], f32r, tag="vacc")

                def _vprod(k):
                    vt = vtmppool.tile([P, S], f32, tag="vtmp")
                    nc.scalar.activation(
                        vt[:], xbf[:, k, :],
                        mybir.ActivationFunctionType.Copy,
                        scale=wvs_sb[:, k : k + 1],
                    )
                    return vt

                p0, p1 = _vprod(0), _vprod(1)
                nc.vector.tensor_tensor(p0[:], p0[:], p1[:], mybir.AluOpType.add)
                p2, p3 = _vprod(2), _vprod(3)
                nc.gpsimd.tensor_tensor(p2[:], p2[:], p3[:], mybir.AluOpType.add)
                nc.gpsimd.tensor_tensor(p0[:], p0[:], p2[:], mybir.AluOpType.add)
                p4, p5 = _vprod(4), _vprod(5)
                nc.vector.tensor_tensor(p4[:], p4[:], p5[:], mybir.AluOpType.add)
                p6, p7 = _vprod(6), _vprod(7)
                nc.gpsimd.tensor_tensor(p6[:], p6[:], p7[:], mybir.AluOpType.add)
                nc.vector.tensor_tensor(p4[:], p4[:], p6[:], mybir.AluOpType.add)
                nc.vector.tensor_tensor(vacc[:], p0[:], p4[:], mybir.AluOpType.add)

                # ---- xpb = bf16(x) + b_proj (residual + bias, per chunk) ----
                xpb = xpbpool.tile([P, KC, S], bf16, tag="xpb")
                for k in range(KC):
                    nc.vector.tensor_scalar(
                        xpb[:, k, :], xbf[:, k, :], bp_sb[:, k : k + 1], None,
                        mybir.AluOpType.add,
                    )

                # ---- stage A: y = (16 M^T) x via fp8 DoubleRow ----
                y_sb = ypool.tile([P, KC, S], bf16, tag="y")
                for n in range(NCH):
                    for mc in range(KC):
                        ps = psA.tile([P, NN], f32, tag="psA")
                        for q in range(QC):
                            nc.tensor.matmul(
                                ps[:],
                                a16_sb[:, q, :, mc, :],
                                x8t[:, q, :, n * NN : (n + 1) * NN],
                                start=(q == 0),
                                stop=(q == QC - 1),
                                perf_mode=mybir.MatmulPerfMode.DoubleRow,
                            )
                        nc.any.tensor_copy(
                            out=y_sb[:, mc, n * NN : (n + 1) * NN], in_=ps[:]
                        )

                # ---- vs broadcast via ones-matmul ----
                vsb = vpool.tile([P, S], f32, tag="vsb")
                for n in range(NCH):
                    psv = psA.tile([P, NN], f32, tag="psA")
                    nc.tensor.matmul(
                        psv[:], ones_sb[:],
                        vacc[:, n * NN : (n + 1) * NN],
                        start=True, stop=True,
                    )
                    nc.any.tensor_copy(out=vsb[:, n * NN : (n + 1) * NN], in_=psv[:])

                if b == 0:
                    # proj weights not needed until stage C
                    for tt in range(TC):
                        nc.sync.dma_start(wpt_sb[:, tt, :], wpt_d[tt])

                # ---- stage B: logits tiles, exp -> bf16 e, row sums ----
                ebf = epool.tile([P, TC, S], bf16, tag="e")
                rsh = [[None] * NCH for _ in range(TC)]
                for n in range(NCH):
                    for tt in range(TC):
                        psl = psB.tile([P, NN], f32, tag="psB")
                        for k in range(KC):
                            nc.tensor.matmul(
                                psl[:],
                                xbf[:, k, tt * P : (tt + 1) * P],
                                y_sb[:, k, n * NN : (n + 1) * NN],
                                start=(k == 0),
                                stop=(k == KC - 1),
                            )
                        rs = spool.tile([P, 1], f32, tag="rs")
                        nc.scalar.activation(
                            ebf[:, tt, n * NN : (n + 1) * NN], psl[:],
                            mybir.ActivationFunctionType.Exp,
                            scale=float(SCALE), accum_out=rs[:],
                        )
                        rsh[tt][n] = rs

                # ---- fold softmax normalization into proj weights ----
                wpts = wptspool.tile([P, TC, C], bf16, tag="wpts")
                for tt in range(TC):
                    rst = spool.tile([P, 1], f32, tag="rst")
                    nc.vector.tensor_tensor(
                        rst[:], rsh[tt][0][:], rsh[tt][1][:], mybir.AluOpType.add
                    )
                    rcp = spool.tile([P, 1], f32, tag="rcp")
                    nc.vector.reciprocal(rcp[:], rst[:])
                    nc.scalar.activation(
                        wpts[:, tt, :], wpt_sb[:, tt, :],
                        mybir.ActivationFunctionType.Copy,
                        scale=rcp[:],
                    )

                # ---- stage C: out = (wpts @ e) * vs + (x + b) ----
                cpools = (
                    [(psC, "psC"), (psA, "psA"), (psB, "psB")]
                    if b == bpc - 1
                    else [(psC, "psC")]
                )
                for oc in range(KC):
                    for n in range(NCH):
                        cp, ctag = cpools[(oc * NCH + n) % len(cpools)]
                        pso = cp.tile([P, NN], f32, tag=ctag)
                        for tt in range(TC):
                            nc.tensor.matmul(
                                pso[:],
                                wpts[:, tt, oc * P : (oc + 1) * P],
                                ebf[:, tt, n * NN : (n + 1) * NN],
                                start=(tt == 0),
                                stop=(tt == TC - 1),
                            )
                        osb = opool.tile([P, NN], f32, tag="osb")
                        nc.vector.tensor_tensor(
                            osb[:], pso[:], vsb[:, n * NN : (n + 1) * NN],
                            mybir.AluOpType.mult,
                        )
                        nc.gpsimd.tensor_tensor(
                            osb[:], osb[:], xpb[:, oc, n * NN : (n + 1) * NN],
                            mybir.AluOpType.add,
                        )
                        nc.sync.dma_start(
                            out_d[b, oc * P : (oc + 1) * P, n * NN : (n + 1) * NN],
                            osb[:],
                        )
    nc.compile()
    return nc


def _host_prep(w_qkv, w_proj, b_proj):
    wq = w_qkv[0:C].astype(np.float64)
    wk = w_qkv[C : 2 * C].astype(np.float64)
    wv = w_qkv[2 * C : 3 * C]
    # lhsT for y-matmul: a16[d, c] = 16*M[c, d], M = Wq^T Wk => a16 = 16*Wk^T Wq
    a16 = np.clip(A_SCALE * (wk.T @ wq), -240.0, 240.0).astype(
        ml_dtypes.float8_e4m3
    )
    # stripe layout [mc][p][q][i][m]: contraction d = q*256 + i*128 + p,
    # output col index c = mc*128 + m
    a16_s = np.ascontiguousarray(
        a16.reshape(QC, 2, P, KC, P).transpose(3, 2, 0, 1, 4)
    )
    wvs = wv.sum(axis=0, dtype=np.float64).astype(np.float32)
    # wpt[tt][p][o] = w_proj[o, t = tt*128 + p]
    wpt_s = np.ascontiguousarray(
        w_proj.T.reshape(TC, P, C).astype(ml_dtypes.bfloat16)
    )
    return a16_s, wpt_s, wvs, b_proj.astype(np.float32)


_NC_CACHE = {}


def _get_nc(bpc=BPC):
    if bpc not in _NC_CACHE:
        _NC_CACHE[bpc] = build_nc(bpc)
    return _NC_CACHE[bpc]


def kernel(x, w_qkv, w_proj, b_proj, _trace=False):
    x = np.asarray(x, dtype=np.float32)
    a16, wpt, wvs, bp = _host_prep(
        np.asarray(w_qkv, np.float32),
        np.asarray(w_proj, np.float32),
        np.asarray(b_proj, np.float32),
    )
    xr_full = x.reshape(B, C, S)
    # fp8 DR layout [b, q, p, i, s]: c = q*256 + i*128 + p
    x8_full = (
        np.clip(xr_full, -240.0, 240.0)
        .astype(ml_dtypes.float8_e4m3)
        .reshape(B, QC, 2, P, S)
        .transpose(0, 1, 3, 2, 4)
    )
    xbf_full = xr_full.astype(ml_dtypes.bfloat16).reshape(B, KC, P, S)
    in_maps = []
    for c in range(N_CORES):
        sl = slice(c * BPC, (c + 1) * BPC)
        in_maps.append(
            {
                "x8": np.ascontiguousarray(x8_full[sl]),
                "xbf": np.ascontiguousarray(xbf_full[sl]),
                "a16": a16,
                "wpt": wpt,
                "wvs": wvs,
                "ones": np.ones((P, P), np.float32),
                "bp": bp,
            }
        )
    nc = _get_nc(BPC)
    res = run_bass_kernel_spmd(
        nc, in_maps, core_ids=list(range(N_CORES)), trace=_trace
    )
    out = np.concatenate([r["out"] for r in res.results], axis=0)
    out = out.reshape(B, C, HH, WW)
    if _trace:
        kernel.last_results = res
    return out
